# revision 4
# baseline (speedup 1.0000x reference)
"""Trainium2 Bass kernel for ViT window attention with relative position bias.

v3: host-side pre-transposition + DMA batching.
Host feeds packed DRAM layouts so phase 1 needs only ~14 large DMAs
(each dma_start costs ~2us serialized on its issuing queue):
  - xp   [128, 4*8*394]   bf16  x^T packed (tci, ct, tok)-major
  - wp   [128, 24576]     bf16  qkv_w^T packed: q/k in (og, ct, 256) blocks,
                                v in (voc, ct, 512) blocks
  - pwp  [128, 8192]      bf16  proj_w^T packed (oc, ct, 512)
  - eba/ebb               bf16  exp(bias), key-major, packed per head
  - bias [128, 2056]      f32   [qb_col | vb broadcast | pb broadcast]
Device graph is a pure matmul pipeline; elementwise work balanced across
vector/scalar/gpsimd (gpsimd cannot touch PSUM).
"""

import os
import sys

for _p in ("/opt/trn_rl_repo", "/root/.axon_site/_ro/trn_rl_repo"):
    if os.path.isdir(_p) and _p not in sys.path:
        sys.path.insert(0, _p)

import numpy as np
import ml_dtypes

import concourse.bass as bass
import concourse.mybir as mybir
import concourse.tile as tile
from concourse import bacc
from concourse.bass import AP

F32 = mybir.dt.float32
BF16 = mybir.dt.bfloat16
AF = mybir.ActivationFunctionType

WIN = 14
NSEQ = WIN * WIN + 1          # 197
H = 16                        # heads
HD = 64                       # head dim
C = 1024
B_FULL = 64
BC = 8                        # batches per core
T = BC * NSEQ                 # 1576 tokens per core
SCALE = HD ** -0.5            # 0.125
TCH = 394                     # B1 token chunk (4 * 394 = 1576)
NT_TILE = 13                  # ceil(1576 / 128)

XP_W = 4 * 8 * TCH            # 12608
WP_W = 8 * 2048 + 2 * 4096    # 24576


def build_nc(stage: int = 99):
    nc = _build_graph(stage)
    nc.compile()
    return nc


def _build_graph(stage: int = 99):
    nc = bacc.Bacc(None)

    def dump(nc, out_ext, ap, row0):
        nc.gpsimd.dma_start(out_ext[row0: row0 + ap.shape[0], 0: ap.free_size()], ap)

    xp_ext = nc.declare_dram_parameter("xp", [128, XP_W], BF16, isOutput=False)
    wp_ext = nc.declare_dram_parameter("wp", [128, WP_W], BF16, isOutput=False)
    pwp_ext = nc.declare_dram_parameter("pwp", [128, 8192], BF16, isOutput=False)
    eba_ext = nc.declare_dram_parameter("eba", [128, H * NSEQ], BF16,
                                        isOutput=False)
    ebb_ext = nc.declare_dram_parameter("ebb", [69, H * NSEQ], BF16,
                                        isOutput=False)
    qb_ext = nc.declare_dram_parameter("qb", [128, 8], F32, isOutput=False)
    vbpb_ext = nc.declare_dram_parameter("vbpb", [128, 2048], F32,
                                         isOutput=False)
    out_ext = nc.declare_dram_parameter("out", [T, C], F32, isOutput=True)

    with tile.TileContext(nc) as tc:
        with tc.tile_pool(name="persist", bufs=1) as pp:
            onesK = pp.tile([128, HD], BF16, name="onesK", tag="onesK")
            nc.gpsimd.memset(onesK[:], 1.0)
            EBA = pp.tile([128, H * NSEQ], BF16, name="EBA", tag="EBA")
            EBB = pp.tile([69, H * NSEQ], BF16, name="EBB", tag="EBB")
            QB = pp.tile([128, 8], F32, name="QB", tag="QB")
            VBPB = pp.tile([128, 2048], F32, name="VBPB", tag="VBPB")
            nc.gpsimd.dma_start(QB[:], qb_ext[:])

            def qb_col(ot):
                return QB[:, ot: ot + 1]

            def vb_sl(ksz, voc):
                return VBPB[0:ksz, voc * 512:(voc + 1) * 512]

            def pb_sl(tsz, oc):
                return VBPB[0:tsz, 1024 + oc * 512: 1024 + (oc + 1) * 512]

            QKT = [
                pp.tile([128, T], BF16, name=f"qkt{ot}", tag=f"qkt{ot}")
                for ot in range(16)
            ]
            V1 = {}
            for b in range(BC):
                V1[(b, 0)] = pp.tile([128, C], BF16,
                                     name=f"v1_{b}_0", tag=f"v1_{b}_0")
                V1[(b, 1)] = pp.tile([69, C], BF16,
                                     name=f"v1_{b}_1", tag=f"v1_{b}_1")

            from contextlib import ExitStack
            outer_ctx = ExitStack()
            xv = outer_ctx.enter_context(tc.tile_pool(name="xv", bufs=1))
            XT = xv.tile([128, XP_W], BF16, name="XT", tag="XT")
            WV = xv.tile([128, 8192], BF16, name="WV", tag="WV")
            with (
                tc.tile_pool(name="wqk", bufs=1) as xw,
                tc.tile_pool(name="psum_mm", bufs=8, space="PSUM") as psmm,
            ):
                WTL = xw.tile([128, 16384], BF16, name="WTL", tag="WTL")

                def x_sl(ct, tci):
                    o = (tci * 8 + ct) * TCH
                    return XT[:, o: o + TCH]

                def x_tok(ct, t0, ksz):
                    tci, toff = divmod(t0, TCH)
                    o = tci * 8 * TCH + ct * TCH + toff
                    return XT[:, o: o + ksz]

                def w_qk(ct, ot):
                    o = (ot // 2) * 2048 + ct * 256 + (ot % 2) * 128
                    return WTL[:, o: o + 128]

                def w_v(ct, voc):
                    o = voc * 4096 + ct * 512
                    return WV[:, o: o + 512]

                # batched loads, ring-ordered by first use.
                # sync ring: x0, x2, x3, EBA, EBB; scalar ring: og0, og1,
                # x1, og2..7, v0, v1, vbpb (pwp issued at phase-2 start).
                def ld_x(tci, eng):
                    eng.dma_start(
                        XT[:, tci * 3152:(tci + 1) * 3152],
                        xp_ext[:, tci * 3152:(tci + 1) * 3152],
                    )

                def ld_og(og):
                    nc.scalar.dma_start(
                        WTL[:, og * 2048:(og + 1) * 2048],
                        wp_ext[:, og * 2048:(og + 1) * 2048],
                    )

                # first chunks split into ct-halves so B1 can start on
                # channels 0-3 while 4-7 are still in flight
                nc.sync.dma_start(XT[:, 0:1576], xp_ext[:, 0:1576])
                nc.scalar.dma_start(WTL[:, 0:1024], wp_ext[:, 0:1024])
                nc.sync.dma_start(XT[:, 1576:3152], xp_ext[:, 1576:3152])
                nc.scalar.dma_start(WTL[:, 1024:2048], wp_ext[:, 1024:2048])
                ld_og(1)
                ld_x(1, nc.sync)
                ld_x(2, nc.scalar)
                ld_x(3, nc.scalar)
                nc.sync.dma_start(EBA[:], eba_ext[:])
                nc.sync.dma_start(EBB[:], ebb_ext[:])
                for og in range(2, 8):
                    ld_og(og)
                for voc in range(2):
                    nc.scalar.dma_start(
                        WV[:, voc * 4096:(voc + 1) * 4096],
                        wp_ext[:, 16384 + voc * 4096: 16384 + (voc + 1) * 4096],
                    )
                nc.scalar.dma_start(VBPB[:], vbpb_ext[:])

                # PE warm-up: ~80 dummy matmuls so the HAM clock gate opens
                # before the first real B1 matmul's operands land.
                wup = psmm.tile([128, 512], F32, name="wup", tag="pq")
                for _ in range(140):
                    nc.tensor.matmul(wup[0:64, 0:64], onesK[:], onesK[:],
                                     start=True, stop=True)

                # --- B1: Q,K projection into QKT (transposed layout) -------
                def b1_chunk(ot, tci):
                    t0 = tci * TCH
                    pq = psmm.tile([128, 512], F32, name="pq", tag="pq")
                    for ct in range(8):
                        nc.tensor.matmul(
                            pq[:, 0:TCH],
                            w_qk(ct, ot),
                            x_sl(ct, tci),
                            start=(ct == 0), stop=(ct == 7),
                        )
                    if ot < 8:
                        nc.vector.tensor_scalar_add(
                            QKT[ot][:, t0: t0 + TCH], pq[:, 0:TCH], qb_col(ot),
                        )
                    else:
                        nc.scalar.copy(QKT[ot][:, t0: t0 + TCH], pq[:, 0:TCH])

                for tci in range(4):
                    for ot in range(4):
                        b1_chunk(ot, tci)
                for ot in range(4, 16):
                    for tci in range(4):
                        b1_chunk(ot, tci)

                if stage <= 2:
                    dump(nc, out_ext, QKT[0][:, 0:1024], 0)
                    dump(nc, out_ext, QKT[8][:, 0:1024], 128)
                    return nc

            # --- V projection -> V1[(b,kt)] bf16 (+bias); b0/b1 now,
            #     b2..b7 interleaved into the attention loop ---
            psv = outer_ctx.enter_context(
                tc.tile_pool(name="psum_v", bufs=2, space="PSUM"))

            def emit_v_group(b, kt, voc):
                ko, ksz = ((0, 128), (128, 69))[kt]
                t0 = b * NSEQ + ko
                pv = psv.tile([128, 512], F32, name="pv", tag="pv")
                for ct in range(8):
                    nc.tensor.matmul(
                        pv[0:ksz, :],
                        x_tok(ct, t0, ksz),
                        w_v(ct, voc),
                        start=(ct == 0), stop=(ct == 7),
                    )
                nc.vector.tensor_add(
                    V1[(b, kt)][0:ksz, voc * 512:(voc + 1) * 512],
                    pv[0:ksz, :],
                    vb_sl(ksz, voc),
                )

            v_jobs = [(b, kt, voc) for b in range(BC)
                      for kt in range(2) for voc in range(2)]
            vctr = [0]

            def maybe_v(n=1):
                while n > 0 and vctr[0] < len(v_jobs):
                    emit_v_group(*v_jobs[vctr[0]])
                    vctr[0] += 1
                    n -= 1

            maybe_v(8)  # b0, b1 complete before attention bp0

            if stage <= 3:
                for b in range(2):
                    dump(nc, out_ext, V1[(b, 0)][:, 0:1024], b * 128)
                return nc

            # ----- phase 2: attention + output projection -------------------
            with (
                tc.tile_pool(name="attbig", bufs=1) as ab,
                tc.tile_pool(name="attw", bufs=8) as attw,
                tc.tile_pool(name="attsmall", bufs=6) as attsmall,
                tc.tile_pool(name="ostage", bufs=3) as op_,
            ):
                att_ctx = ExitStack()
                ps_s = att_ctx.enter_context(
                    tc.tile_pool(name="ps_s", bufs=2, space="PSUM"))
                ps_o = att_ctx.enter_context(
                    tc.tile_pool(name="ps_o", bufs=2, space="PSUM"))
                ps_r = att_ctx.enter_context(
                    tc.tile_pool(name="ps_r", bufs=2, space="PSUM"))
                ps_j = psv
                attnT = [
                    ab.tile([128, T], BF16, name=f"at{ct}", tag=f"at{ct}")
                    for ct in range(8)
                ]
                PWTL = ab.tile([128, 8192], BF16, name="PWTL", tag="PWTL")
                nc.scalar.dma_start(PWTL[:], pwp_ext[:])

                def pw_sl(ct, oc):
                    o = oc * 4096 + ct * 512
                    return PWTL[:, o: o + 512]

                def emit_b4_group(tt, oc, store_eng=None):
                    tsz = min(128, T - tt * 128)
                    pp2 = ps_j.tile([128, 512], F32, name="ppj", tag="pv")
                    for ct in range(8):
                        nc.tensor.matmul(
                            pp2[0:tsz, :],
                            attnT[ct][:, tt * 128: tt * 128 + tsz],
                            pw_sl(ct, oc),
                            start=(ct == 0), stop=(ct == 7),
                        )
                    ost = op_.tile([128, 512], F32, name="ost", tag="ost")
                    nc.vector.tensor_add(
                        ost[0:tsz, :],
                        pp2[0:tsz, :],
                        pb_sl(tsz, oc),
                    )
                    (store_eng or nc.sync).dma_start(
                        out_ext[tt * 128: tt * 128 + tsz,
                                oc * 512:(oc + 1) * 512],
                        ost[0:tsz, :],
                    )

                def emit_b4(tt):
                    emit_b4_group(tt, 0)
                    emit_b4_group(tt, 1)

                b4ctr = [0]   # counts emitted b4 (tt, oc) groups
                b4ready = [0]

                def filler_one(bp):
                    # one dense 8-matmul group: pending b4 first, else a V
                    # group (keeping V two batch-pairs ahead of attention)
                    if b4ctr[0] < b4ready[0]:
                        emit_b4_group(b4ctr[0] // 2, b4ctr[0] % 2)
                        b4ctr[0] += 1
                    elif bp < 3 and vctr[0] < 8 * (bp + 2):
                        maybe_v(1)

                # attention: head-paired column-tiled PV/normalize
                for bp in range(BC // 2):
                    t0p = 2 * bp * NSEQ
                    for hp in range(8):
                        po = ps_o.tile([128, 2 * NSEQ], F32, name="po",
                                       tag="po", padded_shape=[128, 512])
                        rbp = ps_r.tile([128, 2 * NSEQ], F32, name="rbp",
                                        tag="rbp", padded_shape=[128, 512])
                        for hh in range(2):
                            h = 2 * hp + hh
                            qpo = 64 * hh
                            cpos = 64 * hh
                            kot = 8 + hp
                            pss = []
                            for kt, (ko, ksz) in enumerate(((0, 128),
                                                            (128, 69))):
                                ps = ps_s.tile([128, 2 * NSEQ], F32,
                                               name="ps", tag="ps")
                                for bi in range(2):
                                    b = 2 * bp + bi
                                    t0 = b * NSEQ
                                    nc.tensor.matmul(
                                        ps[0:ksz, bi * NSEQ:(bi + 1) * NSEQ],
                                        QKT[kot][qpo: qpo + 64,
                                                 t0 + ko: t0 + ko + ksz],
                                        QKT[hp][qpo: qpo + 64, t0: t0 + NSEQ],
                                        start=True, stop=True,
                                    )
                                pss.append(ps)
                            # dense filler hides the exp/mult latency the
                            # rowsum below depends on
                            filler_one(bp)
                            pts = []
                            for kt, (ko, ksz) in enumerate(((0, 128),
                                                            (128, 69))):
                                pt = attw.tile([128, 2 * NSEQ], BF16,
                                               name="pt", tag="pt")
                                nc.scalar.activation(
                                    pt[0:ksz, :], pss[kt][0:ksz, :], AF.Exp,
                                    scale=SCALE,
                                )
                                # one multiply over both batches: eb repeated
                                # along free dim via stride-0 AP; kt0 on
                                # vector, kt1 on gpsimd so they overlap
                                eba_ap = EBA[:] if kt == 0 else EBB[:]
                                meng = nc.vector if kt == 0 else nc.gpsimd
                                meng.tensor_mul(
                                    pt[0:ksz, :],
                                    pt[0:ksz, :],
                                    AP(eba_ap.tensor,
                                       eba_ap.offset + h * NSEQ,
                                       [[H * NSEQ, ksz], [0, 2], [1, NSEQ]]),
                                )
                                pts.append((pt, ksz))
                            for kt, (ko, ksz) in enumerate(((0, 128),
                                                            (128, 69))):
                                nc.tensor.matmul(
                                    rbp[cpos: cpos + 64, :],
                                    onesK[0:ksz, :], pts[kt][0][0:ksz, :],
                                    start=(kt == 0), stop=(kt == 1),
                                    tile_position=(0, cpos),
                                )
                            for bi in range(2):
                                b = 2 * bp + bi
                                for kt, (ko, ksz) in enumerate(((0, 128),
                                                                (128, 69))):
                                    nc.tensor.matmul(
                                        po[cpos: cpos + 64,
                                           bi * NSEQ:(bi + 1) * NSEQ],
                                        V1[(b, kt)][0:ksz,
                                                    h * HD:(h + 1) * HD],
                                        pts[kt][0][0:ksz,
                                                   bi * NSEQ:(bi + 1) * NSEQ],
                                        start=(kt == 0), stop=(kt == 1),
                                        tile_position=(0, cpos),
                                    )
                        rbs = attsmall.tile([128, 2 * NSEQ], F32,
                                            name="rbs", tag="rbs")
                        nc.vector.reciprocal_approx_fast(rbs[:], rbp[:])
                        nc.vector.tensor_mul(
                            attnT[hp][:, t0p: t0p + 2 * NSEQ],
                            po[:],
                            rbs[:],
                        )
                    b4ready[0] = 2 * (((bp + 1) * 2 * NSEQ) // 128)
                    if bp < 3:
                        while vctr[0] < 8 * (bp + 2):
                            maybe_v(1)

                if stage <= 5:
                    dump(nc, out_ext, attnT[0][:, 0:1024], 0)
                    return nc

                # flush remaining output-projection tiles, stores
                # alternating sync/scalar to overlap completion latency
                while b4ctr[0] < 2 * NT_TILE:
                    eng = nc.scalar if b4ctr[0] % 2 == 0 else nc.sync
                    emit_b4_group(b4ctr[0] // 2, b4ctr[0] % 2, eng)
                    b4ctr[0] += 1
                att_ctx.close()
            outer_ctx.close()

    return nc


_NC = None
LAST_RESULT = None


def _get_nc():
    global _NC
    if _NC is None:
        _NC = build_nc()
    return _NC


def _build_rel_pos_index():
    coords = np.stack(np.meshgrid(np.arange(WIN), np.arange(WIN),
                                  indexing='ij'))
    cf = coords.reshape(2, -1)
    rel = (cf[:, :, None] - cf[:, None, :]).transpose(1, 2, 0).astype(np.int64)
    rel[:, :, 0] += WIN - 1
    rel[:, :, 1] += WIN - 1
    rel[:, :, 0] *= 2 * WIN - 1
    nrel = (2 * WIN - 1) * (2 * WIN - 1) + 3
    idx = np.zeros((NSEQ, NSEQ), dtype=np.int64)
    idx[1:, 1:] = rel.sum(-1)
    idx[0, :] = nrel - 3
    idx[:, 0] = nrel - 2
    idx[0, 0] = nrel - 1
    return idx


def make_in_maps(x, qkv_w, q_bias, v_bias, rpb_table, proj_w, proj_b,
                 rel_pos_index=None):
    bf = ml_dtypes.bfloat16
    x = np.asarray(x, np.float32)
    wt = np.asarray(qkv_w, np.float32).T.astype(bf)            # [1024, 3072]
    wqk = np.ascontiguousarray(
        wt[:, :2048].reshape(8, 128, 8, 256).transpose(1, 2, 0, 3)
    ).reshape(128, 16384)
    wv = np.ascontiguousarray(
        wt[:, 2048:].reshape(8, 128, 2, 512).transpose(1, 2, 0, 3)
    ).reshape(128, 8192)
    wp = np.ascontiguousarray(np.concatenate([wqk, wv], axis=1))

    pwt = np.asarray(proj_w, np.float32).T.astype(bf)          # [1024, 1024]
    pwp = np.ascontiguousarray(
        pwt.reshape(8, 128, 2, 512).transpose(1, 2, 0, 3)
    ).reshape(128, 8192)

    qb_col = np.asarray(q_bias, np.float32).reshape(8, 128).T  # [128, 8]
    vb_bc = np.broadcast_to(
        np.asarray(v_bias, np.float32).reshape(1, C), (128, C))
    pb_bc = np.broadcast_to(
        np.asarray(proj_b, np.float32).reshape(1, C), (128, C))
    qb = np.ascontiguousarray(qb_col)                          # [128, 8]
    vbpb = np.ascontiguousarray(
        np.concatenate([vb_bc, pb_bc], axis=1))                # [128, 2048]

    if rel_pos_index is None:
        rel_pos_index = _build_rel_pos_index()
    idx = np.asarray(rel_pos_index).astype(np.int64)           # [197, 197]
    E = np.exp(np.asarray(rpb_table, np.float32))[idx]         # [q, k, H]
    EBt = E.transpose(2, 1, 0).astype(bf)                      # [H, k, q]
    eba = np.ascontiguousarray(
        EBt[:, 0:128, :].transpose(1, 0, 2)).reshape(128, H * NSEQ)
    ebb = np.ascontiguousarray(
        EBt[:, 128:NSEQ, :].transpose(1, 0, 2)).reshape(69, H * NSEQ)

    in_maps = []
    for c in range(8):
        xt = x[c * BC:(c + 1) * BC].reshape(T, C).T.astype(bf)  # [1024, 1576]
        xpk = np.ascontiguousarray(
            xt.reshape(8, 128, 4, TCH).transpose(1, 2, 0, 3)
        ).reshape(128, XP_W)
        in_maps.append({
            "xp": xpk, "wp": wp, "pwp": pwp,
            "eba": eba, "ebb": ebb, "qb": qb, "vbpb": vbpb,
        })
    return in_maps


def _ensure_axon_hooks_importable():
    """bass_utils imports antenv.axon_hooks when BASS_TRACE is set; the image's
    antenv lacks that module. Provide a no-op stand-in so tracing degrades
    gracefully instead of crashing (unless a real one is already installed)."""
    import types
    try:
        import antenv.axon_hooks  # noqa: F401
    except Exception:
        mod = types.ModuleType("antenv.axon_hooks")
        mod._h = None
        mod.set_axon_ntff_profile_hook = lambda h: setattr(mod, "_h", h)
        mod.get_axon_ntff_profile_hook = lambda: mod._h
        sys.modules["antenv.axon_hooks"] = mod
        try:
            import antenv
            antenv.axon_hooks = mod
        except Exception:
            pass


def kernel(x, qkv_w, q_bias, v_bias, rpb_table, proj_w, proj_b,
           rel_pos_index=None, **_unused):
    global LAST_RESULT
    _ensure_axon_hooks_importable()
    from concourse.bass_utils import run_bass_kernel_spmd

    nc = _get_nc()
    in_maps = make_in_maps(x, qkv_w, q_bias, v_bias, rpb_table, proj_w,
                           proj_b, rel_pos_index)
    res = run_bass_kernel_spmd(nc, in_maps, core_ids=list(range(8)))
    LAST_RESULT = res
    out = np.concatenate(
        [res.results[c]["out"].reshape(BC, NSEQ, C) for c in range(8)], axis=0
    )
    return out.astype(np.float32)
